# revision 24
# baseline (speedup 1.0000x reference)
"""Trainium2 Bass kernel for ExampleGuidedAttention (N=8, C=256, H=W=64).

Data-parallel over batch N across 8 NeuronCores; each core computes one
batch element's full guided attention.

Algorithm notes (per core):
  q = conv_w @ src_pix                      [64, 4096]   (PE, bf16)
  S^T[j,i] = sum_o q[o,j] q[o,i]            (PE, bf16; S symmetric; two
             j-blocks packed in the 128x128 array via tile_position
             (row groups 0-63 / 64-127) since the contraction is only 64)
  F[j,i] = exp(S^T[j,i] - 64)               (ACT; global shift keeps fp32
             exp in range -- softmax ratio unchanged; diag scores are
             chi2(64) so they reach ~120).  The ACT accumulator
             (accum_out) yields Z partials for free: Z[j] = sum_i F[j,i]
             equals the softmax denominator because S is symmetric.
  O[c,i] = sum_j pixT[j,c] * F[j,i]         (PE, bf16, natural layout)
  out    = [ (1-m)*ref_att*invZ + m*ref ; src_att*invZ ]

Performance structure (vs the v1 kernel):
  - inputs land as 8KB-contiguous partition lines ([128, 2048] convert
    chunks): ~2x the bandwidth of the 2KB-line version.  NOTE: 16KB
    lines silently corrupt (descriptor length-field limit) -- keep
    convert-DMA lines at <= 8KB.
  - all [128, HW] column-broadcasts (mask, 1/Z) are ones-vector matmuls
    on the PE into PSUM -- the partition_broadcast DMA path is
    software-dynamic and costs ~25us per 2MB.
  - Z comes free from the exp pass via the ACT accumulator.
  - finalize is restructured as  flow = [(1-m)*ra]*izb + m*ref  with
    (1-m) folded into the PSUM->SBUF copy-out of each slice and m*ref
    precomputed during the input phase, so the tail after the last
    apply matmul is only slice 7's small DVE ops + 4 small DMAs.
  - scores for slice s+1 are issued before apply(s) on a double-buffered
    F ring: the PE never waits on the ACT exp stream.
  - everything downstream of PSUM is bf16 (output DRAM tensor too;
    host casts back to f32) -- halves DVE and output-DMA cost.
"""

import numpy as np

import concourse.bass as bass
import concourse.mybir as mybir
import concourse.tile as tile
from concourse import bacc, bass_utils
from concourse.bass import ts
from concourse.masks import make_identity

P = 128
C = 256          # feature channels
CQ = 64          # query channels
HW = 4096        # pixels per image
NB = HW // P     # 32 pixel blocks (contraction chunks)
SLICE = 512
NS = HW // SLICE  # 8 output column slices
NCORES = 8

F32 = mybir.dt.float32
BF16 = mybir.dt.bfloat16
EXP = mybir.ActivationFunctionType.Exp
COPY = mybir.ActivationFunctionType.Copy
AX_X = mybir.AxisListType.X


def _build_body(tc, src, ref, mask, wT, out, dbg=None):
    nc = tc.nc
    src_r = src.ap().rearrange("(ci p) j -> p ci j", p=P)   # [128, 2, 4096]
    ref_r = ref.ap().rearrange("(ci p) j -> p ci j", p=P)
    wT_r = wT.ap().rearrange("(ci p) o -> p ci o", p=P)     # [128, 2, 64]
    out_r = out.ap().rearrange("(cb p) j -> cb p j", p=P)   # [4, 128, 4096]

    with (
        tc.tile_pool(name="persist", bufs=1) as persist,
        tc.tile_pool(name="ps_s", bufs=4, space="PSUM") as ps_s,
        tc.tile_pool(name="ps_o", bufs=4, space="PSUM") as ps_o,
    ):
        # q duplicated into both partition halves so scores matmuls can be
        # row-packed: tile at rows 0-63 and rows 64-127 run concurrently.
        q2 = persist.tile([P, HW], BF16)
        pixT_src = persist.tile([P, NB, C], BF16)
        pixT_ref = persist.tile([P, NB, C], BF16)
        wT_sb = persist.tile([P, 2, CQ], BF16)
        zpart = persist.tile([P, NB, NS], F32)   # ACT accum of each exp
        z_all = persist.tile([P, NB], F32)
        invz = persist.tile([P, NB], F32)
        onem = persist.tile([P, HW], BF16)       # (1 - mask) broadcast
        m_rep = persist.tile([P, HW], BF16)      # mask broadcast
        vmask = persist.tile([P, 2, HW], BF16)   # mask * ref  (blend addend)
        izb = persist.tile([P, HW], BF16)        # 1/Z broadcast
        o_sb = persist.tile([P, 4, HW], BF16)
        tmp7 = persist.tile([P, SLICE], BF16)
        exp_bias = persist.tile([P, 1], F32)
        ident = persist.tile([P, P], F32)
        invz_T = persist.tile([NB, P], F32)
        ones_st = persist.tile([1, P], BF16)     # stationary ones row (K=1)
        mask_sb = persist.tile([1, HW], BF16)    # mask as a single row
        zrowb = persist.tile([1, HW], BF16)      # 1/Z as a single row
        nc.vector.memset(exp_bias, -64.0)
        nc.vector.memset(ones_st, 1.0)
        make_identity(nc, ident)

        # mask row: 4 tiny cast-DMAs on the gpsimd software path (16KB
        # total, lands in a couple of us before the ref bulk begins)
        for h in range(4):
            jh = slice(h * (HW // 4), (h + 1) * (HW // 4))
            nc.gpsimd.dma_start(
                out=mask_sb[:, jh], in_=mask.ap()[jh].partition_broadcast(1)
            )
        nc.sync.dma_start(out=wT_sb, in_=wT_r)

        with tc.tile_pool(name="early", bufs=1) as early:
            srcb = early.tile([P, 2, HW], BF16)
            refb = early.tile([P, 2, HW], BF16)

            # PE warmup: back-to-back matmuls on zeroed data latch the HAM
            # clock gate up while input DMAs stream in.
            warm_sb = early.tile([P, SLICE], BF16)
            nc.vector.memset(warm_sb, 0.0)
            warm_ps = ps_s.tile([P, SLICE], F32, name="warm_ps", tag="pss")
            for _ in range(20):
                nc.tensor.matmul(
                    warm_ps, warm_sb[:, 0:P], warm_sb, start=True, stop=True
                )

            # input casts on the gpsimd path ([128, 2048] chunks, 8KB
            # lines, ~200 GB/s = the per-core DRAM read floor).  h-major
            # order so each chunk immediately enables its transpose and
            # conv quarter.  The XBAR transposes ride sync/scalar:
            # pixT[p, b, c] = pix[c, b*128+p]
            JH = HW // 2
            for h in range(2):
                jh = slice(h * JH, (h + 1) * JH)
                bh = slice(h * (NB // 2), (h + 1) * (NB // 2))
                for ci in range(2):
                    cs = slice(ci * P, (ci + 1) * P)
                    nc.gpsimd.dma_start(out=srcb[:, ci, jh], in_=src_r[:, ci, jh])
                    nc.sync.dma_start_transpose(
                        out=pixT_src[:, bh, cs], in_=srcb[:, ci, jh]
                    )
            for h in range(2):
                jh = slice(h * JH, (h + 1) * JH)
                bh = slice(h * (NB // 2), (h + 1) * (NB // 2))
                for ci in range(2):
                    cs = slice(ci * P, (ci + 1) * P)
                    nc.gpsimd.dma_start(out=refb[:, ci, jh], in_=ref_r[:, ci, jh])
                    nc.scalar.dma_start_transpose(
                        out=pixT_ref[:, bh, cs], in_=refb[:, ci, jh]
                    )

            # mask column-broadcast via ones-vector matmuls (PE); ACT
            # drains each PSUM tile into (1-m) and m right away
            for s in range(NS):
                sl = ts(s, SLICE)
                psm = ps_s.tile([P, SLICE], F32, name="psm", tag="pss")
                nc.tensor.matmul(
                    psm, ones_st, mask_sb[:, sl], start=True, stop=True
                )
                nc.scalar.activation(
                    out=onem[:, sl], in_=psm, func=COPY, bias=1.0, scale=-1.0
                )
                nc.scalar.activation(out=m_rep[:, sl], in_=psm, func=COPY)

            # 1x1 conv: q = wT.T @ src_pix; q into both partition halves
            for s in range(NS):
                sl = ts(s, SLICE)
                psq = ps_s.tile([CQ, SLICE], F32, name="psq", tag="pss")
                for ci in range(2):
                    nc.tensor.matmul(
                        psq,
                        wT_sb[:, ci, :],
                        srcb[:, ci, sl],
                        start=(ci == 0),
                        stop=(ci == 1),
                    )
                nc.vector.tensor_copy(out=q2[0:CQ, sl], in_=psq)
                nc.vector.tensor_copy(out=q2[CQ:P, sl], in_=psq)

            # vmask = m * ref (bf16, consumed only by the finalize adds)
            for ci in range(2):
                for s in range(NS):
                    sl = ts(s, SLICE)
                    nc.vector.tensor_mul(
                        vmask[:, ci, sl], m_rep[:, sl], refb[:, ci, sl]
                    )

        def scores_and_exp(s, f_sb):
            sl = ts(s, SLICE)
            for jp in range(NB // 2):
                jb0, jb1 = 2 * jp, 2 * jp + 1
                pss0 = ps_s.tile([P, SLICE], F32, name="pss0", tag="pss")
                pss1 = ps_s.tile([P, SLICE], F32, name="pss1", tag="pss")
                nc.tensor.matmul(
                    pss0, q2[0:CQ, ts(jb0, P)], q2[0:CQ, sl],
                    start=True, stop=True, tile_position=(0, 0),
                )
                nc.tensor.matmul(
                    pss1, q2[CQ:P, ts(jb1, P)], q2[CQ:P, sl],
                    start=True, stop=True, tile_position=(CQ, 0),
                )
                for jb, pss in ((jb0, pss0), (jb1, pss1)):
                    nc.scalar.activation(
                        out=f_sb[:, jb, :], in_=pss, func=EXP, bias=exp_bias,
                        accum_out=zpart[:, jb, s : s + 1],
                    )

        def apply_mm(s, f_sb, mid_hook=None):
            psos = [
                ps_o.tile([P, SLICE], F32, name=f"pso{cb}", tag="pso")
                for cb in range(4)
            ]
            for jb in range(NB):
                if jb == 10 and mid_hook is not None:
                    mid_hook()
                for cb in range(4):
                    pt = pixT_src if cb < 2 else pixT_ref
                    lhs = pt[:, jb, (cb % 2) * P : (cb % 2 + 1) * P]
                    nc.tensor.matmul(
                        psos[cb], lhs, f_sb[:, jb, :],
                        start=(jb == 0), stop=(jb == NB - 1),
                    )
            return psos

        def copy_out(s, psos):
            # src_att: plain PSUM->SBUF copy; ref_att: fold (1-m) in
            sl = ts(s, SLICE)
            nc.vector.tensor_copy(out=o_sb[:, 0, sl], in_=psos[0])
            nc.vector.tensor_copy(out=o_sb[:, 1, sl], in_=psos[1])
            nc.vector.tensor_mul(o_sb[:, 2, sl], psos[2], onem[:, sl])
            nc.vector.tensor_mul(o_sb[:, 3, sl], psos[3], onem[:, sl])

        def izb_broadcast():
            # 1/Z row -> [128, HW] via ones-vector matmuls; ACT drains PSUM
            # (the DVE is busy with finalize at this point)
            for s2 in range(NS):
                sl2 = ts(s2, SLICE)
                psz = ps_s.tile([P, SLICE], F32, name="psz", tag="pss")
                nc.tensor.matmul(
                    psz, ones_st, zrowb[:, sl2], start=True, stop=True
                )
                nc.scalar.activation(out=izb[:, sl2], in_=psz, func=COPY)

        def finalize(lo, hi, dma_engines):
            """Normalize + blend + store for pixel columns [lo:hi)."""
            r = slice(lo, hi)
            for ci in range(2):
                nc.vector.tensor_mul(o_sb[:, 2 + ci, r], o_sb[:, 2 + ci, r], izb[:, r])
                nc.vector.tensor_add(o_sb[:, 2 + ci, r], o_sb[:, 2 + ci, r], vmask[:, ci, r])
                nc.vector.tensor_mul(o_sb[:, ci, r], o_sb[:, ci, r], izb[:, r])
            # out rows: [flow(=cb2,3), src_att(=cb0,1)]
            for k, cb in enumerate([2, 3, 0, 1]):
                eng = dma_engines[k % len(dma_engines)]
                eng.dma_start(out=out_r[k, :, r], in_=o_sb[:, cb, r])

        with tc.tile_pool(name="fbuf", bufs=2) as fbuf:
            # double-buffered F ring: exp(s+1) writes one buffer while
            # apply(s) streams the other, so scores stay one slice ahead
            f_cur = fbuf.tile([P, NB, SLICE], BF16, name="f_sb", tag="f")
            scores_and_exp(0, f_cur)
            for s in range(NS - 2):
                f_next = fbuf.tile([P, NB, SLICE], BF16, name="f_sb", tag="f")
                scores_and_exp(s + 1, f_next)
                psos = apply_mm(s, f_cur)
                copy_out(s, psos)
                f_cur = f_next
            s7 = NS - 1
            f_sb = fbuf.tile([P, NB, SLICE], BF16, name="f_sb", tag="f")
            scores_and_exp(s7, f_sb)
            # Z partials all land once exp(s7) retires (during apply(s6));
            # issuing the DVE reduce before copy_out(s6) lets it run early
            nc.vector.reduce_sum(out=z_all, in_=zpart, axis=AX_X)
            nc.vector.reciprocal(out=invz, in_=z_all)
            psos = apply_mm(s7 - 1, f_cur)
            copy_out(s7 - 1, psos)
            ps_t = ps_s.tile([NB, P], F32, name="ps_t", tag="pss")
            nc.tensor.transpose(ps_t, invz[:, :], ident)
            nc.vector.tensor_copy(out=invz_T, in_=ps_t)
            # flatten [32 partitions, 128] -> one [1, 4096] row (SBUF->SBUF
            # DMA crosses partitions; f32 -> bf16 converts on the way)
            nc.gpsimd.dma_start(
                out=zrowb.rearrange("a (b q) -> a b q", q=P), in_=invz_T
            )
            # slice 7 apply; after 10 j-blocks the zrowb row has landed, so
            # the izb broadcast matmuls slot into the middle of the stream
            psos7 = apply_mm(s7, f_sb, mid_hook=izb_broadcast)
            finalize(0, (NS - 1) * SLICE, [nc.sync, nc.scalar, nc.gpsimd])
            # slice 7: copy-out doubles as normalize + blend
            sl7 = ts(s7, SLICE)
            nc.vector.tensor_mul(tmp7, onem[:, sl7], izb[:, sl7])
            nc.vector.tensor_mul(o_sb[:, 0, sl7], psos7[0], izb[:, sl7])
            nc.vector.tensor_mul(o_sb[:, 1, sl7], psos7[1], izb[:, sl7])
            for ci in range(2):
                nc.vector.tensor_mul(o_sb[:, 2 + ci, sl7], psos7[2 + ci], tmp7)
                nc.vector.tensor_add(
                    o_sb[:, 2 + ci, sl7], o_sb[:, 2 + ci, sl7], vmask[:, ci, sl7]
                )
            for k, cb in enumerate([2, 3, 0, 1]):
                eng = [nc.sync, nc.scalar, nc.gpsimd, nc.sync][k]
                eng.dma_start(out=out_r[k, :, sl7], in_=o_sb[:, cb, sl7])

            if dbg is not None:
                nc.sync.dma_start(out=dbg["q2"].ap(), in_=q2)
                nc.sync.dma_start(
                    out=dbg["zpart"].ap().rearrange("p (b s) -> p b s", s=NS),
                    in_=zpart,
                )
                nc.sync.dma_start(out=dbg["invz"].ap(), in_=invz)
                nc.sync.dma_start(out=dbg["izb"].ap(), in_=izb)
                nc.sync.dma_start(out=dbg["onem"].ap(), in_=onem)
                nc.sync.dma_start(
                    out=dbg["vmask"].ap().rearrange("p (ci j) -> p ci j", ci=2),
                    in_=vmask,
                )
                nc.sync.dma_start(
                    out=dbg["f7"].ap().rearrange("p (b i) -> p b i", b=NB),
                    in_=f_sb,
                )


def build():
    nc = bacc.Bacc(
        "TRN2",
        target_bir_lowering=False,
        debug=False,
        enable_asserts=False,
        num_devices=NCORES,
    )
    src = nc.dram_tensor("src", (C, HW), F32, kind="ExternalInput")
    ref = nc.dram_tensor("ref", (C, HW), F32, kind="ExternalInput")
    mask = nc.dram_tensor("mask", (HW,), F32, kind="ExternalInput")
    wT = nc.dram_tensor("wT", (C, CQ), BF16, kind="ExternalInput")
    out = nc.dram_tensor("out", (2 * C, HW), BF16, kind="ExternalOutput")
    with tile.TileContext(nc) as tc:
        _build_body(tc, src, ref, mask, wT, out)
    nc.compile()
    return nc


_CACHE = {}


def _get_nc():
    if "nc" not in _CACHE:
        _CACHE["nc"] = build()
    return _CACHE["nc"]


def _in_maps(src_mask, src_feature, ref_feature, conv_w):
    import ml_dtypes

    n_batch = src_feature.shape[0]
    wT = np.ascontiguousarray(
        np.asarray(conv_w, dtype=np.float32).T.astype(ml_dtypes.bfloat16)
    )
    maps = []
    for n in range(n_batch):
        maps.append(
            {
                "src": np.ascontiguousarray(
                    np.asarray(src_feature[n], dtype=np.float32).reshape(C, HW)
                ),
                "ref": np.ascontiguousarray(
                    np.asarray(ref_feature[n], dtype=np.float32).reshape(C, HW)
                ),
                "mask": np.ascontiguousarray(
                    np.asarray(src_mask[n], dtype=np.float32).reshape(HW)
                ),
                "wT": wT,
            }
        )
    return maps


def _install_ntff_hook():
    """The agent image's antenv lacks axon_hooks; recreate it so
    run_bass_kernel_spmd(trace=True) can capture NTFF profiles."""
    import sys
    import types

    if "antenv.axon_hooks" in sys.modules:
        return
    import antenv
    from trn_agent_boot.trn_boot import _ntff_profile_via_ctypes

    hook = _ntff_profile_via_ctypes("/opt/axon/libaxon_pjrt.so")
    mod = types.ModuleType("antenv.axon_hooks")
    mod._hook = hook
    mod.set_axon_ntff_profile_hook = lambda h: setattr(mod, "_hook", h)
    mod.get_axon_ntff_profile_hook = lambda: mod._hook
    sys.modules["antenv.axon_hooks"] = mod
    antenv.axon_hooks = mod


def run(src_mask, src_feature, ref_feature, conv_w, trace=False):
    """Run on 8 NeuronCores. Returns (output [N,2C,H,W], BassKernelResults)."""
    n_batch, c, h, w = src_feature.shape
    if trace:
        _install_ntff_hook()
    nc = _get_nc()
    maps = _in_maps(src_mask, src_feature, ref_feature, conv_w)
    res = bass_utils.run_bass_kernel_spmd(
        nc, maps, core_ids=list(range(NCORES)), trace=trace
    )
    out = np.stack([np.asarray(r["out"]) for r in res.results], axis=0)
    return out.reshape(n_batch, 2 * c, h, w).astype(np.float32), res


def kernel(src_mask, src_feature, ref_feature, conv_w):
    out, _ = run(src_mask, src_feature, ref_feature, conv_w)
    return out


# revision 26
# speedup vs baseline: 1.0038x; 1.0038x over previous
"""Trainium2 Bass kernel for ExampleGuidedAttention (N=8, C=256, H=W=64).

Data-parallel over batch N across 8 NeuronCores; each core computes one
batch element's full guided attention.

Algorithm notes (per core):
  q = conv_w @ src_pix                      [64, 4096]   (PE, bf16)
  S^T[j,i] = sum_o q[o,j] q[o,i]            (PE, bf16; S symmetric; two
             j-blocks packed in the 128x128 array via tile_position
             (row groups 0-63 / 64-127) since the contraction is only 64)
  F[j,i] = exp(S^T[j,i] - 64)               (ACT; global shift keeps fp32
             exp in range -- softmax ratio unchanged; diag scores are
             chi2(64) so they reach ~120).  The ACT accumulator
             (accum_out) yields Z partials for free: Z[j] = sum_i F[j,i]
             equals the softmax denominator because S is symmetric.
  O[c,i] = sum_j pixT[j,c] * F[j,i]         (PE, bf16, natural layout)
  out    = [ (1-m)*ref_att*invZ + m*ref ; src_att*invZ ]

Performance structure (vs the v1 kernel):
  - inputs land as 8KB-contiguous partition lines ([128, 2048] convert
    chunks): ~2x the bandwidth of the 2KB-line version.  NOTE: 16KB
    lines silently corrupt (descriptor length-field limit) -- keep
    convert-DMA lines at <= 8KB.
  - all [128, HW] column-broadcasts (mask, 1/Z) are ones-vector matmuls
    on the PE into PSUM -- the partition_broadcast DMA path is
    software-dynamic and costs ~25us per 2MB.
  - Z comes free from the exp pass via the ACT accumulator.
  - finalize is restructured as  flow = [(1-m)*ra]*izb + m*ref  with
    (1-m) folded into the PSUM->SBUF copy-out of each slice and m*ref
    precomputed during the input phase, so the tail after the last
    apply matmul is only slice 7's small DVE ops + 4 small DMAs.
  - scores for slice s+1 are issued before apply(s) on a double-buffered
    F ring: the PE never waits on the ACT exp stream.
  - everything downstream of PSUM is bf16 (output DRAM tensor too;
    host casts back to f32) -- halves DVE and output-DMA cost.
"""

import numpy as np

import concourse.bass as bass
import concourse.mybir as mybir
import concourse.tile as tile
from concourse import bacc, bass_utils
from concourse.bass import ts
from concourse.masks import make_identity

P = 128
C = 256          # feature channels
CQ = 64          # query channels
HW = 4096        # pixels per image
NB = HW // P     # 32 pixel blocks (contraction chunks)
SLICE = 512
NS = HW // SLICE  # 8 output column slices
NCORES = 8

F32 = mybir.dt.float32
BF16 = mybir.dt.bfloat16
EXP = mybir.ActivationFunctionType.Exp
COPY = mybir.ActivationFunctionType.Copy
AX_X = mybir.AxisListType.X


def _build_body(tc, src, ref, mask, wT, out, dbg=None):
    nc = tc.nc
    src_r = src.ap().rearrange("(ci p) j -> p ci j", p=P)   # [128, 2, 4096]
    ref_r = ref.ap().rearrange("(ci p) j -> p ci j", p=P)
    wT_r = wT.ap().rearrange("(ci p) o -> p ci o", p=P)     # [128, 2, 64]
    out_r = out.ap().rearrange("(cb p) j -> cb p j", p=P)   # [4, 128, 4096]

    with (
        tc.tile_pool(name="persist", bufs=1) as persist,
        tc.tile_pool(name="ps_s", bufs=4, space="PSUM") as ps_s,
        tc.tile_pool(name="ps_o", bufs=4, space="PSUM") as ps_o,
    ):
        # q duplicated into both partition halves so scores matmuls can be
        # row-packed: tile at rows 0-63 and rows 64-127 run concurrently.
        q2 = persist.tile([P, HW], BF16)
        pixT_src = persist.tile([P, NB, C], BF16)
        pixT_ref = persist.tile([P, NB, C], BF16)
        wT_sb = persist.tile([P, 2, CQ], BF16)
        zpart = persist.tile([P, NB, NS], F32)   # ACT accum of each exp
        z_all = persist.tile([P, NB], F32)
        invz = persist.tile([P, NB], F32)
        onem = persist.tile([P, HW], BF16)       # (1 - mask) broadcast
        m_rep = persist.tile([P, HW], BF16)      # mask broadcast
        vmask = persist.tile([P, 2, HW], BF16)   # mask * ref  (blend addend)
        izb = persist.tile([P, HW], BF16)        # 1/Z broadcast
        o_sb = persist.tile([P, 4, HW], BF16)
        tmp7 = persist.tile([P, SLICE], BF16)
        exp_bias = persist.tile([P, 1], F32)
        ident = persist.tile([P, P], F32)
        invz_T = persist.tile([NB, P], F32)
        ones_st = persist.tile([1, P], BF16)     # stationary ones row (K=1)
        mask_sb = persist.tile([1, HW], BF16)    # mask as a single row
        zrowb = persist.tile([1, HW], BF16)      # 1/Z as a single row
        nc.vector.memset(exp_bias, -64.0)
        nc.vector.memset(ones_st, 1.0)
        make_identity(nc, ident)

        # mask row: 4 tiny cast-DMAs on the gpsimd software path (16KB
        # total, lands in a couple of us before the ref bulk begins)
        for h in range(4):
            jh = slice(h * (HW // 4), (h + 1) * (HW // 4))
            nc.gpsimd.dma_start(
                out=mask_sb[:, jh], in_=mask.ap()[jh].partition_broadcast(1)
            )
        nc.sync.dma_start(out=wT_sb, in_=wT_r)

        with tc.tile_pool(name="early", bufs=1) as early:
            srcb = early.tile([P, 2, HW], BF16)
            refb = early.tile([P, 2, HW], BF16)

            # PE warmup: back-to-back matmuls on zeroed data latch the HAM
            # clock gate up while input DMAs stream in.
            warm_sb = early.tile([P, SLICE], BF16)
            nc.vector.memset(warm_sb, 0.0)
            warm_ps = ps_s.tile([P, SLICE], F32, name="warm_ps", tag="pss")
            for _ in range(20):
                nc.tensor.matmul(
                    warm_ps, warm_sb[:, 0:P], warm_sb, start=True, stop=True
                )

            # input casts on the gpsimd path ([128, 2048] chunks, 8KB
            # lines; the DMA fabric tops out ~200 GB/s per core, shared
            # with the XBAR transposes below).  h-major so conv quarters
            # unblock early.
            JH = HW // 2
            for h in range(2):
                jh = slice(h * JH, (h + 1) * JH)
                for ci in range(2):
                    nc.gpsimd.dma_start(out=srcb[:, ci, jh], in_=src_r[:, ci, jh])
            for h in range(2):
                jh = slice(h * JH, (h + 1) * JH)
                for ci in range(2):
                    nc.gpsimd.dma_start(out=refb[:, ci, jh], in_=ref_r[:, ci, jh])

            # XBAR transposes (sync: src, scalar: ref), issued after all
            # input kicks: pixT[p, b, c] = pix[c, b*128+p]
            for h in range(2):
                jh = slice(h * JH, (h + 1) * JH)
                bh = slice(h * (NB // 2), (h + 1) * (NB // 2))
                for ci in range(2):
                    cs = slice(ci * P, (ci + 1) * P)
                    nc.sync.dma_start_transpose(
                        out=pixT_src[:, bh, cs], in_=srcb[:, ci, jh]
                    )
                    nc.scalar.dma_start_transpose(
                        out=pixT_ref[:, bh, cs], in_=refb[:, ci, jh]
                    )

            # mask column-broadcast via ones-vector matmuls (PE); ACT
            # drains each PSUM tile into (1-m) and m right away
            for s in range(NS):
                sl = ts(s, SLICE)
                psm = ps_s.tile([P, SLICE], F32, name="psm", tag="pss")
                nc.tensor.matmul(
                    psm, ones_st, mask_sb[:, sl], start=True, stop=True
                )
                nc.scalar.activation(
                    out=onem[:, sl], in_=psm, func=COPY, bias=1.0, scale=-1.0
                )
                nc.scalar.activation(out=m_rep[:, sl], in_=psm, func=COPY)

            # 1x1 conv: q = wT.T @ src_pix; q into both partition halves
            for s in range(NS):
                sl = ts(s, SLICE)
                psq = ps_s.tile([CQ, SLICE], F32, name="psq", tag="pss")
                for ci in range(2):
                    nc.tensor.matmul(
                        psq,
                        wT_sb[:, ci, :],
                        srcb[:, ci, sl],
                        start=(ci == 0),
                        stop=(ci == 1),
                    )
                nc.vector.tensor_copy(out=q2[0:CQ, sl], in_=psq)
                nc.vector.tensor_copy(out=q2[CQ:P, sl], in_=psq)

            # vmask = m * ref (bf16, consumed only by the finalize adds)
            for ci in range(2):
                for s in range(NS):
                    sl = ts(s, SLICE)
                    nc.vector.tensor_mul(
                        vmask[:, ci, sl], m_rep[:, sl], refb[:, ci, sl]
                    )

        def scores_and_exp(s, f_sb):
            sl = ts(s, SLICE)
            for jp in range(NB // 2):
                jb0, jb1 = 2 * jp, 2 * jp + 1
                pss0 = ps_s.tile([P, SLICE], F32, name="pss0", tag="pss")
                pss1 = ps_s.tile([P, SLICE], F32, name="pss1", tag="pss")
                nc.tensor.matmul(
                    pss0, q2[0:CQ, ts(jb0, P)], q2[0:CQ, sl],
                    start=True, stop=True, tile_position=(0, 0),
                )
                nc.tensor.matmul(
                    pss1, q2[CQ:P, ts(jb1, P)], q2[CQ:P, sl],
                    start=True, stop=True, tile_position=(CQ, 0),
                )
                for jb, pss in ((jb0, pss0), (jb1, pss1)):
                    nc.scalar.activation(
                        out=f_sb[:, jb, :], in_=pss, func=EXP, bias=exp_bias,
                        accum_out=zpart[:, jb, s : s + 1],
                    )

        def apply_mm(s, f_sb, mid_hook=None):
            # cb-major: the src channel blocks stream first, so slice 0's
            # apply can begin before the ref transposes have landed
            psos = [
                ps_o.tile([P, SLICE], F32, name=f"pso{cb}", tag="pso")
                for cb in range(4)
            ]
            for cb in range(4):
                if cb == 1 and mid_hook is not None:
                    mid_hook()
                pt = pixT_src if cb < 2 else pixT_ref
                for jb in range(NB):
                    lhs = pt[:, jb, (cb % 2) * P : (cb % 2 + 1) * P]
                    nc.tensor.matmul(
                        psos[cb], lhs, f_sb[:, jb, :],
                        start=(jb == 0), stop=(jb == NB - 1),
                    )
            return psos

        def copy_out(s, psos):
            # src_att: plain PSUM->SBUF copy; ref_att: fold (1-m) in
            sl = ts(s, SLICE)
            nc.vector.tensor_copy(out=o_sb[:, 0, sl], in_=psos[0])
            nc.vector.tensor_copy(out=o_sb[:, 1, sl], in_=psos[1])
            nc.vector.tensor_mul(o_sb[:, 2, sl], psos[2], onem[:, sl])
            nc.vector.tensor_mul(o_sb[:, 3, sl], psos[3], onem[:, sl])

        def izb_broadcast():
            # 1/Z row -> [128, HW] via ones-vector matmuls; ACT drains PSUM
            # (the DVE is busy with finalize at this point)
            for s2 in range(NS):
                sl2 = ts(s2, SLICE)
                psz = ps_s.tile([P, SLICE], F32, name="psz", tag="pss")
                nc.tensor.matmul(
                    psz, ones_st, zrowb[:, sl2], start=True, stop=True
                )
                nc.scalar.activation(out=izb[:, sl2], in_=psz, func=COPY)

        def finalize(lo, hi, dma_engines):
            """Normalize + blend + store for pixel columns [lo:hi)."""
            r = slice(lo, hi)
            for ci in range(2):
                nc.vector.tensor_mul(o_sb[:, 2 + ci, r], o_sb[:, 2 + ci, r], izb[:, r])
                nc.vector.tensor_add(o_sb[:, 2 + ci, r], o_sb[:, 2 + ci, r], vmask[:, ci, r])
                nc.vector.tensor_mul(o_sb[:, ci, r], o_sb[:, ci, r], izb[:, r])
            # out rows: [flow(=cb2,3), src_att(=cb0,1)]
            for k, cb in enumerate([2, 3, 0, 1]):
                eng = dma_engines[k % len(dma_engines)]
                eng.dma_start(out=out_r[k, :, r], in_=o_sb[:, cb, r])

        with tc.tile_pool(name="fbuf", bufs=2) as fbuf:
            # double-buffered F ring: exp(s+1) writes one buffer while
            # apply(s) streams the other, so scores stay one slice ahead
            f_cur = fbuf.tile([P, NB, SLICE], BF16, name="f_sb", tag="f")
            scores_and_exp(0, f_cur)
            for s in range(NS - 2):
                f_next = fbuf.tile([P, NB, SLICE], BF16, name="f_sb", tag="f")
                scores_and_exp(s + 1, f_next)
                psos = apply_mm(s, f_cur)
                copy_out(s, psos)
                f_cur = f_next
            s7 = NS - 1
            f_sb = fbuf.tile([P, NB, SLICE], BF16, name="f_sb", tag="f")
            scores_and_exp(s7, f_sb)
            # Z partials all land once exp(s7) retires (during apply(s6));
            # issuing the DVE reduce before copy_out(s6) lets it run early
            nc.vector.reduce_sum(out=z_all, in_=zpart, axis=AX_X)
            nc.vector.reciprocal(out=invz, in_=z_all)
            psos = apply_mm(s7 - 1, f_cur)
            copy_out(s7 - 1, psos)
            ps_t = ps_s.tile([NB, P], F32, name="ps_t", tag="pss")
            nc.tensor.transpose(ps_t, invz[:, :], ident)
            nc.vector.tensor_copy(out=invz_T, in_=ps_t)
            # flatten [32 partitions, 128] -> one [1, 4096] row (SBUF->SBUF
            # DMA crosses partitions; f32 -> bf16 converts on the way)
            nc.gpsimd.dma_start(
                out=zrowb.rearrange("a (b q) -> a b q", q=P), in_=invz_T
            )
            # slice 7 apply; after 10 j-blocks the zrowb row has landed, so
            # the izb broadcast matmuls slot into the middle of the stream
            psos7 = apply_mm(s7, f_sb, mid_hook=izb_broadcast)
            finalize(0, (NS - 1) * SLICE, [nc.sync, nc.scalar, nc.gpsimd])
            # slice 7: copy-out doubles as normalize + blend
            sl7 = ts(s7, SLICE)
            nc.vector.tensor_mul(tmp7, onem[:, sl7], izb[:, sl7])
            nc.vector.tensor_mul(o_sb[:, 0, sl7], psos7[0], izb[:, sl7])
            nc.vector.tensor_mul(o_sb[:, 1, sl7], psos7[1], izb[:, sl7])
            for ci in range(2):
                nc.vector.tensor_mul(o_sb[:, 2 + ci, sl7], psos7[2 + ci], tmp7)
                nc.vector.tensor_add(
                    o_sb[:, 2 + ci, sl7], o_sb[:, 2 + ci, sl7], vmask[:, ci, sl7]
                )
            for k, cb in enumerate([2, 3, 0, 1]):
                eng = [nc.sync, nc.scalar, nc.gpsimd, nc.sync][k]
                eng.dma_start(out=out_r[k, :, sl7], in_=o_sb[:, cb, sl7])

            if dbg is not None:
                nc.sync.dma_start(out=dbg["q2"].ap(), in_=q2)
                nc.sync.dma_start(
                    out=dbg["zpart"].ap().rearrange("p (b s) -> p b s", s=NS),
                    in_=zpart,
                )
                nc.sync.dma_start(out=dbg["invz"].ap(), in_=invz)
                nc.sync.dma_start(out=dbg["izb"].ap(), in_=izb)
                nc.sync.dma_start(out=dbg["onem"].ap(), in_=onem)
                nc.sync.dma_start(
                    out=dbg["vmask"].ap().rearrange("p (ci j) -> p ci j", ci=2),
                    in_=vmask,
                )
                nc.sync.dma_start(
                    out=dbg["f7"].ap().rearrange("p (b i) -> p b i", b=NB),
                    in_=f_sb,
                )


def build():
    nc = bacc.Bacc(
        "TRN2",
        target_bir_lowering=False,
        debug=False,
        enable_asserts=False,
        num_devices=NCORES,
    )
    src = nc.dram_tensor("src", (C, HW), F32, kind="ExternalInput")
    ref = nc.dram_tensor("ref", (C, HW), F32, kind="ExternalInput")
    mask = nc.dram_tensor("mask", (HW,), F32, kind="ExternalInput")
    wT = nc.dram_tensor("wT", (C, CQ), BF16, kind="ExternalInput")
    out = nc.dram_tensor("out", (2 * C, HW), BF16, kind="ExternalOutput")
    with tile.TileContext(nc) as tc:
        _build_body(tc, src, ref, mask, wT, out)
    nc.compile()
    return nc


_CACHE = {}


def _get_nc():
    if "nc" not in _CACHE:
        _CACHE["nc"] = build()
    return _CACHE["nc"]


def _in_maps(src_mask, src_feature, ref_feature, conv_w):
    import ml_dtypes

    n_batch = src_feature.shape[0]
    wT = np.ascontiguousarray(
        np.asarray(conv_w, dtype=np.float32).T.astype(ml_dtypes.bfloat16)
    )
    maps = []
    for n in range(n_batch):
        maps.append(
            {
                "src": np.ascontiguousarray(
                    np.asarray(src_feature[n], dtype=np.float32).reshape(C, HW)
                ),
                "ref": np.ascontiguousarray(
                    np.asarray(ref_feature[n], dtype=np.float32).reshape(C, HW)
                ),
                "mask": np.ascontiguousarray(
                    np.asarray(src_mask[n], dtype=np.float32).reshape(HW)
                ),
                "wT": wT,
            }
        )
    return maps


def _install_ntff_hook():
    """The agent image's antenv lacks axon_hooks; recreate it so
    run_bass_kernel_spmd(trace=True) can capture NTFF profiles."""
    import sys
    import types

    if "antenv.axon_hooks" in sys.modules:
        return
    import antenv
    from trn_agent_boot.trn_boot import _ntff_profile_via_ctypes

    hook = _ntff_profile_via_ctypes("/opt/axon/libaxon_pjrt.so")
    mod = types.ModuleType("antenv.axon_hooks")
    mod._hook = hook
    mod.set_axon_ntff_profile_hook = lambda h: setattr(mod, "_hook", h)
    mod.get_axon_ntff_profile_hook = lambda: mod._hook
    sys.modules["antenv.axon_hooks"] = mod
    antenv.axon_hooks = mod


def run(src_mask, src_feature, ref_feature, conv_w, trace=False):
    """Run on 8 NeuronCores. Returns (output [N,2C,H,W], BassKernelResults)."""
    n_batch, c, h, w = src_feature.shape
    if trace:
        _install_ntff_hook()
    nc = _get_nc()
    maps = _in_maps(src_mask, src_feature, ref_feature, conv_w)
    res = bass_utils.run_bass_kernel_spmd(
        nc, maps, core_ids=list(range(NCORES)), trace=trace
    )
    out = np.stack([np.asarray(r["out"]) for r in res.results], axis=0)
    return out.reshape(n_batch, 2 * c, h, w).astype(np.float32), res


def kernel(src_mask, src_feature, ref_feature, conv_w):
    out, _ = run(src_mask, src_feature, ref_feature, conv_w)
    return out
